# revision 43
# baseline (speedup 1.0000x reference)
"""Tensor-parallel multi-head attention for Trainium2 (8 NeuronCores).

Reference computation (fp32):
    qkv = hidden @ w_qkv.T + b_qkv            # [B,S,3H]
    q,k,v = split/heads                       # [B,NH,S,HD]
    out_h = softmax(q k^T / sqrt(HD)) v       # [B,NH,S,HD]
    out = concat_heads(out_h) @ w_out.T + b_out
Sharding: Megatron-style tensor parallel over NH=16 heads, 2 heads/core;
AllGather of the per-chunk context, disjoint output-column slices per core.

v2 (from trace analysis of the fp32r baseline, 716us):
  - all matmul operands fp16: same 1 cycle/row as fp32r but enables the
    PE's fast weight load (fp32 weights disable FWL), halves SBUF/DMA
    bytes, and halves the AllGather payload (the fp32 gathers caused
    ~90us of PE stalls).  fp16 quantization (2^-11) keeps rel-l2 ~1e-3.
  - softmax denominators: the per-k-tile ones-matmuls (8192 PE rows/head)
    are replaced by fp16 DVE accumulation of the exp tiles (2x DVE mode)
    plus a single 512-row partition-reduce matmul per head.
  - exp activations run on k-tile PAIRS ([128,1024] PSUM reads across two
    banks) - at [128,512] the Scalar engine (680ns/tile) would rate-limit
    the attention phase once the sum-matmuls are gone.
  - the deferred normalization is emitted after the FIRST score pair of
    the next head (not a full head later), so context chunks ship ~7us
    earlier and the tail projections wait less on their gathers.
"""

import sys

sys.path.insert(0, "/opt/trn_rl_repo")

import numpy as np

import concourse.bass as bass
import concourse.tile as tile
from concourse import mybir
from concourse.bass_utils import run_bass_kernel_spmd
from concourse.tile import ScopedClock

FP32 = mybir.dt.float32
F16 = mybir.dt.float16

B = 2
S = 2048
H = 2048
NH = 16
HD = 128
N_CORES = 8
HPC = NH // N_CORES  # heads per core = 2
T = B * S  # 4096
O_QK = 2 * HPC * HD  # 512 rows of qk^T per core (Q then K)
O_V = HPC * HD  # 256
O_OUT = H // N_CORES  # 256 output columns per core
SCALE = 1.0 / float(np.sqrt(HD))
P = 128

MAX_WAITS = 1  # the pinned walrus codegen rejects >1 sync wait per inst


def _wait_limit(inst):
    return MAX_WAITS


class _TileContext(tile.TileContext):
    """Tile patched for the pinned walrus codegen's sync-wait limit.

    Any instruction carrying more than MAX_WAITS semaphore waits is split:
    preceding same-engine nops carry the excess (engines execute their
    stream in order, so the waits still all precede the instruction).
    """

    def _lower_ordered_insts(self, ordered):
        nc = self.nc
        for bb_name, insts in list(ordered.items()):
            new_insts = []
            for inst in insts:
                si = inst.sync_info
                limit = _wait_limit(inst)
                if (
                    si is not None
                    and len(si.on_wait) > limit
                    and inst.engine is not None
                ):
                    waits = list(si.on_wait)
                    while len(waits) > limit:
                        chunk, waits = waits[:limit], waits[limit:]
                        new_insts.append(
                            mybir.InstNoOp(
                                name=nc.get_next_instruction_name(),
                                sync_info=mybir.SyncInfo(
                                    on_wait=chunk, on_update=[]
                                ),
                                bass_nofuse=True,
                                engine=inst.engine,
                            )
                        )
                    inst.sync_info = mybir.SyncInfo(
                        on_wait=waits, on_update=list(si.on_update)
                    )
                new_insts.append(inst)
            ordered[bb_name] = new_insts
        return super()._lower_ordered_insts(ordered)

    def _drain_and_barrier(self, tick_clock, wait_clock):
        nc = self.nc
        probe = nc.sync.nop(nofuse=True, hint="drain_wait_probe")
        wait_clock.add_sem_waits(
            probe.ins, ScopedClock({None: tick_clock.global_clock})
        )
        si = probe.ins.sync_info
        waits = list(si.on_wait) if si is not None else []
        probe.ins.sync_info = mybir.SyncInfo(
            on_wait=[], on_update=list(si.on_update) if si else []
        )
        for w in waits:
            n = nc.sync.nop(nofuse=True, hint="drain_wait_split")
            n.ins.sync_info = mybir.SyncInfo(on_wait=[w], on_update=[])
        nc.sync.drain()
        nc.all_engine_barrier()
        assert self.sems is not None
        popped = nc._tile_sem_poison_stack.pop()
        assert popped is self._sem_poison
        nc.clear_and_free_semaphores(list(self.sems.allocated().values()))
        nc.all_engine_barrier()


def _build_program(seq=S):
    """Build the SPMD Bass program (identical on all 8 cores)."""
    t_all = B * seq
    n_ht = H // P  # 16 k-tiles over the hidden dim
    ts_w = 512  # token-slice width for the QKV stage
    n_ts = t_all // ts_w
    qs_w = 512 if seq % 512 == 0 else 256  # q-slice width in attention
    n_qs = seq // qs_w
    n_kt = seq // P  # k tiles per batch in attention
    n_tt = t_all // P
    n_dt = H // P  # d tiles of the gathered context
    LAG = 2  # chunks of attention emitted before a chunk's projection

    nc = bass.Bass(
        "TRN2", target_bir_lowering=False, debug=False, num_devices=N_CORES
    )

    # pre-tiled on host to [partition, k-tile, free] so each DMA descriptor
    # covers a partition's full contiguous row
    xt = nc.dram_tensor("xt", [P, n_ht, t_all], F16, kind="ExternalInput")
    w1t_qk = nc.dram_tensor(
        "w1t_qk", [P, n_ht, O_QK], F16, kind="ExternalInput"
    )
    w1t_v = nc.dram_tensor("w1t_v", [P, n_ht, O_V], F16, kind="ExternalInput")
    b_qk = nc.dram_tensor("b_qk", [P, O_QK // P], FP32, kind="ExternalInput")
    b_v = nc.dram_tensor("b_v", [P, O_V], FP32, kind="ExternalInput")
    wout_t = nc.dram_tensor(
        "wout_t", [P, n_dt, O_OUT], F16, kind="ExternalInput"
    )
    b_out = nc.dram_tensor("b_out", [P, O_OUT // P], FP32, kind="ExternalInput")
    ones_d = nc.dram_tensor("ones_d", [P, 1], F16, kind="ExternalInput")
    out = nc.dram_tensor("out", [O_OUT, t_all], FP32, kind="ExternalOutput")

    n_ch = B * n_qs  # token chunks, gathered + projected as they finish
    cc_in = nc.dram_tensor("cc_in", [n_ch, O_V, qs_w], F16)
    cc_out = nc.dram_tensor("cc_out", [n_ch, H, qs_w], F16, addr_space="Shared")
    WARM_W = 2048  # 512KB: big enough to warm the collective data path
    warm_in = nc.dram_tensor("warm_in", [P, WARM_W], F16)
    warm_out = nc.dram_tensor(
        "warm_out", [N_CORES * P, WARM_W], F16, addr_space="Shared"
    )

    xt_r = xt.ap()
    w1t_qk_r = w1t_qk.ap()
    w1t_v_r = w1t_v.ap()
    wout_r = wout_t.ap()
    cc_in_r = cc_in.ap().rearrange("c (h p) t -> c p h t", p=P)
    cc_out_r = cc_out.ap().rearrange("c (dt p) t -> c p dt t", p=P)
    out_r = out.ap().rearrange("(ot p) t -> p ot t", p=P)

    MM = nc.tensor.matmul

    lp = nc.allow_low_precision(
        reason="fp16 softmax weights/denominators; validated rel-l2 ~1e-3"
    )
    lp.__enter__()
    with _TileContext(nc) as tc:
        with tc.tile_pool(name="const", bufs=1) as const:
            b_qk_sb = const.tile([P, O_QK // P], FP32)
            nc.sync.dma_start(b_qk_sb[:], b_qk.ap())
            b_v_sb = const.tile([P, O_V], FP32)
            nc.sync.dma_start(b_v_sb[:], b_v.ap())
            b_out_sb = const.tile([P, O_OUT // P], FP32)
            nc.sync.dma_start(b_out_sb[:], b_out.ap())
            ones_col = const.tile([P, 1], F16)
            nc.sync.dma_start(ones_col[:], ones_d.ap())
            ones_row = const.tile([1, P], F16)
            nc.vector.memset(ones_row[:], 1.0)

            # AllGather early in the QKV phase: absorbs the ~11us
            # first-collective setup cost off the critical path
            warm_sb = const.tile([P, WARM_W], F16)
            nc.vector.memset(warm_sb[:], 0.0)
            nc.sync.dma_start(warm_in.ap(), warm_sb[:])
            nc.gpsimd.collective_compute(
                "AllGather",
                mybir.AluOpType.bypass,
                replica_groups=[list(range(N_CORES))],
                ins=[warm_in.ap()],
                outs=[warm_out.ap()],
            )

            # ---------------- Stages 1+2 (qk^T and V resident) ----------
            acts_scope = tc.tile_pool(name="acts", bufs=1)
            acts = acts_scope.__enter__()
            qk_sb = acts.tile([P, O_QK // P, t_all], F16)  # qk^T
            v_sb = acts.tile([P, n_tt, O_V], F16)  # V natural

            # ---------------- Stage 1: QKV projection ----------------
            with (
                tc.tile_pool(name="wq", bufs=1) as wq,
                tc.tile_pool(name="xts", bufs=2) as xts,
                tc.tile_pool(name="ps1", bufs=1, space="PSUM") as ps1,
                tc.tile_pool(name="ps1v", bufs=2, space="PSUM") as ps1v,
            ):
                # chunked weight loads: the first matmuls only wait on the
                # first slice instead of the full weight set.  The first xt
                # k-group is interleaved right after the first weight chunk
                # so the PE starts ~4us after launch.
                WCH = 4  # k-tiles per weight DMA chunk
                w_qk_ch = [
                    wq.tile([P, WCH, O_QK], F16, name=f"w_qk_{i}")
                    for i in range(n_ht // WCH)
                ]
                w_v_ch = [
                    wq.tile([P, WCH, O_V], F16, name=f"w_v_{i}")
                    for i in range(n_ht // WCH)
                ]
                xt_t0 = xts.tile([P, n_ht, ts_w], F16)

                def load_wqk(i):
                    nc.sync.dma_start(
                        w_qk_ch[i][:], w1t_qk_r[:, i * WCH : (i + 1) * WCH, :]
                    )

                def load_xt(t, ts_i, g):
                    nc.sync.dma_start(
                        t[:, g * WCH : (g + 1) * WCH, :],
                        xt_r[
                            :,
                            g * WCH : (g + 1) * WCH,
                            ts_i * ts_w : (ts_i + 1) * ts_w,
                        ],
                    )

                load_wqk(0)
                load_xt(xt_t0, 0, 0)
                for i in range(1, n_ht // WCH):
                    load_wqk(i)
                    load_xt(xt_t0, 0, i)
                for i in range(n_ht // WCH):
                    nc.sync.dma_start(
                        w_v_ch[i][:], w1t_v_r[:, i * WCH : (i + 1) * WCH, :]
                    )

                for ts_i in range(n_ts):
                    if ts_i == 0:
                        xt_t = xt_t0
                    else:
                        xt_t = xts.tile([P, n_ht, ts_w], F16)
                        for g in range(n_ht // WCH):
                            load_xt(xt_t, ts_i, g)
                    # k-group outer / out-tile inner with 4 live accumulators:
                    # the first matmuls only need weight chunk 0 + xt group 0
                    # (1MB) instead of the whole slice (4MB)
                    pss = [
                        ps1.tile([P, ts_w], FP32, name=f"ps_qk{ot}")
                        for ot in range(O_QK // P)
                    ]
                    for g in range(n_ht // WCH):
                        for ot in range(O_QK // P):
                            for k in range(WCH):
                                kt = g * WCH + k
                                MM(
                                    pss[ot][:],
                                    w_qk_ch[g][:, k, ot * P : (ot + 1) * P],
                                    xt_t[:, kt, :],
                                    start=(kt == 0),
                                    stop=(kt == n_ht - 1),
                                )
                    for ot in range(O_QK // P):
                        nc.scalar.activation(
                            qk_sb[:, ot, ts_i * ts_w : (ts_i + 1) * ts_w],
                            pss[ot][:],
                            mybir.ActivationFunctionType.Identity,
                            bias=b_qk_sb[:, ot : ot + 1],
                        )
                    for tt in range(ts_w // P):
                        psv = ps1v.tile([P, O_V], FP32)
                        for kt in range(n_ht):
                            MM(
                                psv[:],
                                xt_t[:, kt, tt * P : (tt + 1) * P],
                                w_v_ch[kt // WCH][:, kt % WCH, :],
                                start=(kt == 0),
                                stop=(kt == n_ht - 1),
                            )
                        nc.vector.tensor_add(
                            v_sb[:, ts_i * (ts_w // P) + tt, :], psv[:], b_v_sb[:]
                        )

            # ------- Stages 2+3 fused: attention -> gather -> projection ----
            # per 512-token chunk: attention for both heads, ship ctx^T via
            # a chunk AllGather, and run that chunk's output projection --
            # collectives and stage-3 DMA overlap later chunks' attention.
            with (
                tc.tile_pool(name="wo", bufs=1) as wo,
                tc.tile_pool(name="ctxp", bufs=3) as ctxp,
                tc.tile_pool(name="exps", bufs=4) as exps,
                tc.tile_pool(name="accs", bufs=3) as accs,
                tc.tile_pool(name="sums", bufs=2) as sums,
                tc.tile_pool(name="ctxs", bufs=2) as ctxs,
                tc.tile_pool(name="outs", bufs=3) as outs,
                tc.tile_pool(name="ps_s", bufs=2, space="PSUM") as ps_s_pool,
                tc.tile_pool(name="ps_c", bufs=2, space="PSUM") as ps_c_pool,
                # ps_sum / ps_b share one bank (tag "ps_r"): strictly
                # sequential lifetimes within a head
                tc.tile_pool(name="ps_r", bufs=1, space="PSUM") as ps_r_pool,
                tc.tile_pool(name="ps_o", bufs=1, space="PSUM") as ps_o_pool,
            ):
                wout_sb = wo.tile([P, n_dt, O_OUT], F16)
                nc.sync.dma_start(wout_sb[:], wout_r)
                sub_w = 512  # stage-3 token sub-chunk (DMA/SBUF granularity)

                def proj_load(ch):
                    # chunked by dt-group: the first proj matmuls of a tail
                    # chunk start ~4.5us earlier (512KB vs 2MB behind them)
                    ctx_t = ctxs.tile([P, n_dt, sub_w], F16, name="ctx_t")
                    for g in range(n_dt // 4):
                        nc.sync.dma_start(
                            ctx_t[:, g * 4 : (g + 1) * 4, :],
                            cc_out_r[ch][:, g * 4 : (g + 1) * 4, :],
                        )
                    return ctx_t

                # Projection micro-ops, interleaved two per attention score
                # pair: the exp ACT (~1.4us/pair) outpaces the pair's 4
                # matmuls (~1.05us), so the proj matmuls ride in the slack
                # and the PE stays the rate limiter.
                proj_q = []
                proj_ps = {}

                def enqueue_proj(ch, ctx_t):
                    for ot in range(O_OUT // P):
                        for dt in range(n_dt):
                            proj_q.append(("mm", ch, ctx_t, ot, dt))
                        proj_q.append(("fin", ch, ot))

                def emit_proj(op):
                    kind, ch, *rest = op
                    b, qs = divmod(ch, n_qs)
                    t_lo = b * seq + qs * qs_w
                    if kind == "mm":
                        ctx_t, ot, dt = rest
                        if dt == 0:
                            proj_ps[(ch, ot)] = ps_o_pool.tile(
                                [P, sub_w], FP32, name="ps_o"
                            )
                        MM(
                            proj_ps[(ch, ot)][:],
                            wout_sb[:, dt, ot * P : (ot + 1) * P],
                            ctx_t[:, dt, :],
                            start=(dt == 0),
                            stop=(dt == n_dt - 1),
                        )
                    else:
                        (ot,) = rest
                        ps_o = proj_ps.pop((ch, ot))
                        out_t = outs.tile([P, sub_w], FP32, name="out_t")
                        nc.scalar.activation(
                            out_t[:],
                            ps_o[:],
                            mybir.ActivationFunctionType.Identity,
                            bias=b_out_sb[:, ot : ot + 1],
                        )
                        nc.sync.dma_start(
                            out_r[:, ot, t_lo : t_lo + sub_w], out_t[:]
                        )

                def pop_proj(n):
                    while n > 0 and proj_q:
                        op = proj_q.pop(0)
                        emit_proj(op)
                        if op[0] == "mm":
                            n -= 1

                def ship_chunk(ch, ctx_ch):
                    nc.sync.dma_start(cc_in_r[ch], ctx_ch[:])
                    nc.gpsimd.collective_compute(
                        "AllGather",
                        mybir.AluOpType.bypass,
                        replica_groups=[list(range(N_CORES))],
                        ins=[cc_in.ap()[ch]],
                        outs=[cc_out.ap()[ch]],
                    )

                def attn_head(ch, h, ctx_ch, pend_a):
                    """Attention for one head, software-pipelined: the score
                    matmuls for pair kp+1 are emitted before the ctx matmuls
                    of pair kp, so the PE never sits behind an exp ACT in its
                    FIFO.  The previous head's normalization is emitted in two
                    deferred stages (denominator reduce at kp1, broadcast +
                    multiply + ship at kp4) so no PE op waits on the ACT/DVE
                    chain."""
                    b, qs = divmod(ch, n_qs)
                    q_lo = b * seq + qs * qs_w
                    q_ap = qk_sb[:, h, q_lo : q_lo + qs_w]
                    ps_ctx = ps_c_pool.tile([P, qs_w], FP32, name="ps_ctx")
                    acc = None
                    n_kp = n_kt // 2

                    def score_pair(kp):
                        ps_pair = ps_s_pool.tile(
                            [P, 2 * qs_w], FP32, name="ps_pair"
                        )
                        for j in range(2):
                            kt = 2 * kp + j
                            k_lo = b * seq + kt * P
                            MM(
                                ps_pair[:, j * qs_w : (j + 1) * qs_w],
                                qk_sb[:, HPC + h, k_lo : k_lo + P],
                                q_ap,
                                start=True,
                                stop=True,
                            )
                        exp_t = exps.tile([P, 2 * qs_w], F16, name="exp_t")
                        nc.scalar.activation(
                            exp_t[:],
                            ps_pair[:],
                            mybir.ActivationFunctionType.Exp,
                            scale=SCALE,
                        )
                        return exp_t

                    def ctx_pair(kp, exp_t):
                        nonlocal acc
                        for j in range(2):
                            kt = 2 * kp + j
                            MM(
                                ps_ctx[:],
                                v_sb[
                                    :, (b * seq) // P + kt, h * HD : (h + 1) * HD
                                ],
                                exp_t[:, j * qs_w : (j + 1) * qs_w],
                                start=(kt == 0),
                                stop=(kt == n_kt - 1),
                            )
                        # denominator: accumulate exp on the DVE (2x fp16
                        # mode); ping-pong buffers keep every op out-of-place
                        if acc is None:
                            acc = accs.tile([P, qs_w], F16, name="acc")
                            nc.vector.tensor_add(
                                acc[:], exp_t[:, :qs_w], exp_t[:, qs_w:]
                            )
                        else:
                            nxt = accs.tile([P, qs_w], F16, name="acc")
                            nc.vector.tensor_add(
                                nxt[:], acc[:], exp_t[:, :qs_w]
                            )
                            acc = nxt
                            nxt = accs.tile([P, qs_w], F16, name="acc")
                            nc.vector.tensor_add(
                                nxt[:], acc[:], exp_t[:, qs_w:]
                            )
                            acc = nxt

                    pend_b = None
                    prev_exp = score_pair(0)
                    pop_proj(2)
                    for kp in range(1, n_kp):
                        exp_t = score_pair(kp)
                        ctx_pair(kp - 1, prev_exp)
                        prev_exp = exp_t
                        pop_proj(2)
                        if kp == 1 and pend_a is not None:
                            pend_b = norm_stage_a(*pend_a)
                            pend_a = None
                        if kp == 4 and pend_b is not None:
                            norm_stage_b(*pend_b)
                            pend_b = None
                    ctx_pair(n_kp - 1, prev_exp)
                    if pend_a is not None:
                        pend_b = norm_stage_a(*pend_a)
                    if pend_b is not None:
                        norm_stage_b(*pend_b)
                    return ps_ctx, acc

                def norm_stage_a(ch, h, ctx_ch, ps_ctx, acc):
                    """Denominator partition-reduce + 1/x via ACT ln->exp
                    (ln and exp share an activation table set, so no table
                    swap; the slow DVE reciprocal is 3.3us)."""
                    ps_sum = ps_r_pool.tile(
                        [P, qs_w], FP32, tag="ps_r", name="ps_sum"
                    )
                    MM(
                        ps_sum[0:1, :], ones_col[:], acc[:],
                        start=True, stop=True,
                    )
                    lns = sums.tile([1, qs_w], FP32, name="lns")
                    nc.scalar.activation(
                        lns[:], ps_sum[0:1, :], mybir.ActivationFunctionType.Ln
                    )
                    inv = sums.tile([1, qs_w], F16, name="inv")
                    nc.scalar.activation(
                        inv[:],
                        lns[:],
                        mybir.ActivationFunctionType.Exp,
                        scale=-1.0,
                    )
                    return (ch, h, ctx_ch, ps_ctx, inv)

                def norm_stage_b(ch, h, ctx_ch, ps_ctx, inv):
                    ps_b = ps_r_pool.tile(
                        [P, qs_w], FP32, tag="ps_r", name="ps_b"
                    )
                    MM(ps_b[:], ones_row[:], inv[:], start=True, stop=True)
                    invb = sums.tile([P, qs_w], F16, name="invb")
                    nc.vector.tensor_copy(invb[:], ps_b[:])
                    nc.vector.tensor_mul(ctx_ch[:, h, :], ps_ctx[:], invb[:])
                    if h == HPC - 1:
                        ship_chunk(ch, ctx_ch)

                ctx_tiles = {}
                proj_tiles = {}
                pend_a = None  # (ch, h, ctx_ch, ps_ctx, acc)
                for ch in range(n_ch):
                    ctx_tiles[ch] = ctxp.tile(
                        [P, HPC, qs_w], F16, name="ctx_ch"
                    )
                    if ch - LAG >= 0:
                        # prefetch the gathered context half a chunk before
                        # its projection matmuls start
                        proj_tiles[ch - LAG] = proj_load(ch - LAG)
                    for h in range(HPC):
                        # proj(ch-LAG) rides the next 32 pair slots (h1 of
                        # this chunk + h0 of the next); the FIRST chunk's
                        # projection starts one head later still, clear of
                        # the slow first gather (~26us vs 19 steady)
                        if h == 1 and ch == LAG + 1:
                            enqueue_proj(0, proj_tiles.pop(0))
                        if h == 1 and ch - LAG >= 1:
                            enqueue_proj(
                                ch - LAG, proj_tiles.pop(ch - LAG)
                            )
                        ps_ctx, acc = attn_head(ch, h, ctx_tiles[ch], pend_a)
                        pend_a = (ch, h, ctx_tiles[ch], ps_ctx, acc)
                # tail: finish the last head's norm (the ship of the final
                # chunk) with leftover proj matmuls filling the ACT latency
                pend_b = norm_stage_a(*pend_a)
                pop_proj(5)
                norm_stage_b(*pend_b)
                tail_tiles = {
                    ch: proj_load(ch) for ch in range(n_ch - LAG, n_ch)
                }
                pop_proj(10**9)
                # dummy matmuls on resident tiles before each tail chunk:
                # they run during the gather wait and keep the PE's HAM
                # clock-gate at 13/16 instead of dropping to 4/8, so the
                # tail projections run at full throttled speed
                warm_ps = ps_r_pool.tile(
                    [P, qs_w], FP32, tag="ps_r", name="warm_ps"
                )
                for n_warm, ch in zip((30, 60), range(n_ch - LAG, n_ch)):
                    for _ in range(n_warm):
                        MM(
                            warm_ps[:],
                            wout_sb[:, 0, 0:P],
                            qk_sb[:, 0, 0:qs_w],
                            start=True,
                            stop=True,
                        )
                    enqueue_proj(ch, tail_tiles[ch])
                    pop_proj(10**9)

            acts_scope.__exit__(None, None, None)

    lp.__exit__(None, None, None)
    return nc


def _tile_rows(a, dtype=np.float16):
    """[H, F] -> [128, H//128, F] (row r = kt*128 + p becomes [p, kt])."""
    h, f = a.shape
    return np.ascontiguousarray(
        a.reshape(h // P, P, f).transpose(1, 0, 2).astype(dtype)
    )


def _make_in_maps(hidden_states, w_qkv, b_qkv, w_out, b_out):
    b, s, _ = hidden_states.shape
    t_all = b * s
    x = _tile_rows(
        np.ascontiguousarray(hidden_states.reshape(t_all, H).T, dtype=np.float32)
    )  # [P, H//P, T] fp16
    in_maps = []
    for c in range(N_CORES):
        h0 = HPC * c
        q_rows = np.r_[h0 * HD : (h0 + HPC) * HD]
        k_rows = H + q_rows
        v_rows = 2 * H + q_rows
        qk_rows = np.r_[q_rows, k_rows]
        w1t_qk = _tile_rows(w_qkv[qk_rows, :].T)
        w1t_v = _tile_rows(w_qkv[v_rows, :].T)
        b_qk = np.ascontiguousarray(
            b_qkv[qk_rows].reshape(O_QK // P, P).T, dtype=np.float32
        )
        b_v = np.ascontiguousarray(
            np.broadcast_to(b_qkv[v_rows], (P, O_V)), dtype=np.float32
        )
        o_lo = c * O_OUT
        wout_t = _tile_rows(w_out[o_lo : o_lo + O_OUT, :].T)
        b_o = np.ascontiguousarray(
            b_out[o_lo : o_lo + O_OUT].reshape(O_OUT // P, P).T,
            dtype=np.float32,
        )
        in_maps.append(
            {
                "ones_d": np.ones((P, 1), dtype=np.float16),
                "xt": x,
                "w1t_qk": w1t_qk,
                "w1t_v": w1t_v,
                "b_qk": b_qk,
                "b_v": b_v,
                "wout_t": wout_t,
                "b_out": b_o,
            }
        )
    return in_maps


_program_cache = {}


def _get_program(seq=S):
    if seq not in _program_cache:
        _program_cache[seq] = _build_program(seq)
    return _program_cache[seq]


def run(hidden_states, w_qkv, b_qkv, w_out, b_out, trace=False):
    """Run the sharded kernel; returns (output, BassKernelResults)."""
    b, s, _ = hidden_states.shape
    nc = _get_program(s)
    in_maps = _make_in_maps(hidden_states, w_qkv, b_qkv, w_out, b_out)
    res = run_bass_kernel_spmd(
        nc, in_maps, list(range(N_CORES)), trace=trace
    )
    # per-core output is out^T [O_OUT, T]; stack to [H, T] then transpose
    cols = np.concatenate([res.results[c]["out"] for c in range(N_CORES)], axis=0)
    return (
        np.ascontiguousarray(cols.T).reshape(b, s, H).astype(np.float32),
        res,
    )


def kernel(hidden_states, w_qkv, b_qkv, w_out, b_out):
    out, _ = run(
        np.asarray(hidden_states),
        np.asarray(w_qkv),
        np.asarray(b_qkv),
        np.asarray(w_out),
        np.asarray(b_out),
    )
    return out
